# revision 1
# baseline (speedup 1.0000x reference)
"""DGCNLayer (layer%2==0 branch) on 8 Trainium2 NeuronCores via Bass.

Math (per reference, with uv_vals == 1 per the problem spec and using
linearity to pull the dense GEMM past the segment-sum):
  User_n = leaky_relu(segsum_{rows}(vfea[cols]) @ W1 + b1, 0.1)
  Item_n = leaky_relu(segsum_{cols}(ufea[rows]) @ W2 + b2, 0.1)
  User_h = relu(concat([ufea, User_n]) @ Wu + bu)
  Item_h = relu(concat([vfea, Item_n]) @ Wi + bi)
  return stack([User_h, User_n, ufea, Item_h, Item_n, vfea])

Distribution: destination nodes are sharded 12500/core across the 8
cores (the sharding hint's destination-partitioned edge lists); the
gather tables (full vfea/ufea) are replicated into every core's HBM so
no collectives are needed. Per core and direction, edges are sorted by
(dst, src) and processed in 128-edge chunks: an indirect DMA gathers
the 128 source rows (one per partition), a one-hot selection matrix S
(built by a single tensor_scalar is_equal against an iota row) maps
edges to the 256 destinations of the current tile, and TensorE
accumulates psum[f, d] += msgs^T @ S over the tile's chunks. The dense
tail (W1/W2 matmul + LeakyReLU, union matmul + ReLU) consumes the
transposed aggregate directly from PSUM-staged SBUF tiles. Outputs are
written feature-major [128, 12544] and the host reassembles the stack.
"""
import sys
sys.path.insert(0, "/opt/trn_rl_repo")
import numpy as np

from concourse import bass, bacc, mybir
from concourse import bass_utils
from concourse.tile import TileContext

F32 = mybir.dt.float32
F32R = mybir.dt.float32r
BF16 = mybir.dt.bfloat16
I32 = mybir.dt.int32

NCORES = 8
N_NODES = 100000
SH = N_NODES // NCORES      # 12500 destinations per core
D = 128
TW = 256                    # dst-tile width (psum half-bank)
AGG_ROWS = 12544            # 12500 padded to x256
NT = AGG_ROWS // TW         # 49 dst tiles
GK = 4                      # 128-edge chunks gathered per indirect DMA
ALPHA = 0.1


def _prep_direction(dst_all: np.ndarray, src_all: np.ndarray):
    """Per-core edge lists sorted by (dst, src), tiled by 256 dsts, each
    tile's count padded to a cross-core-common multiple of 128.

    Returns (srcs, dstf, nchunks): srcs[c] int32 [128, EP/128] wrapped
    gather indices, dstf[c] float32 [128, EP/128] wrapped tile-relative
    dst (pad = -1), nchunks[t] = number of 128-chunks of tile t."""
    cores = []
    for c in range(NCORES):
        m = (dst_all >= c * SH) & (dst_all < (c + 1) * SH)
        d = dst_all[m] - c * SH
        s = src_all[m]
        o = np.lexsort((s, d))
        d, s = d[o], s[o]
        cnt = np.bincount(d // TW, minlength=NT)
        cores.append((d, s, cnt))

    nchunks = []
    for t in range(NT):
        mx = max(int(pc[2][t]) for pc in cores)
        nchunks.append(max(1, -(-mx // 128)))

    srcs, dstf = [], []
    for c in range(NCORES):
        d, s, cnt = cores[c]
        sp, dp = [], []
        off = 0
        for t in range(NT):
            n, p = int(cnt[t]), nchunks[t] * 128
            sk = np.zeros(p, np.int32)
            dk = np.full(p, -1.0, np.float32)
            sk[:n] = s[off:off + n]
            dk[:n] = (d[off:off + n] - t * TW).astype(np.float32)
            sp.append(sk)
            dp.append(dk)
            off += n
        sa = np.concatenate(sp)
        da = np.concatenate(dp)
        srcs.append(sa.reshape(-1, 128).T.copy())   # [128, EP/128]
        dstf.append(da.reshape(-1, 128).T.copy())
    return srcs, dstf, nchunks


def _build(nc: bass.Bass, nch_u, nch_i, epu: int, epi: int):
    vtab = nc.dram_tensor("vtab", [N_NODES, D], BF16, kind="ExternalInput")
    utab = nc.dram_tensor("utab", [N_NODES, D], BF16, kind="ExternalInput")
    ufeaT = nc.dram_tensor("ufeaT", [128, SH], F32, kind="ExternalInput")
    vfeaT = nc.dram_tensor("vfeaT", [128, SH], F32, kind="ExternalInput")
    gsu = nc.dram_tensor("gsu", [128, epu // 128], I32, kind="ExternalInput")
    gdu = nc.dram_tensor("gdu", [128, epu // 128], F32, kind="ExternalInput")
    gsi = nc.dram_tensor("gsi", [128, epi // 128], I32, kind="ExternalInput")
    gdi = nc.dram_tensor("gdi", [128, epi // 128], F32, kind="ExternalInput")
    iota = nc.dram_tensor("iota", [128, TW], F32, kind="ExternalInput")
    wn = {}
    for w in ("W1", "W2", "Wu_t", "Wu_b", "Wi_t", "Wi_b"):
        wn[w] = nc.dram_tensor(w, [128, 128], F32, kind="ExternalInput")
    for b in ("b1", "b2", "bu", "bi"):
        wn[b] = nc.dram_tensor(b, [128, 1], F32, kind="ExternalInput")

    unT = nc.dram_tensor("unT", [128, AGG_ROWS], F32, kind="ExternalOutput")
    uhT = nc.dram_tensor("uhT", [128, AGG_ROWS], F32, kind="ExternalOutput")
    inT = nc.dram_tensor("inT", [128, AGG_ROWS], F32, kind="ExternalOutput")
    ihT = nc.dram_tensor("ihT", [128, AGG_ROWS], F32, kind="ExternalOutput")

    with TileContext(nc) as tc:
        with (
            tc.tile_pool(name="wts", bufs=1) as wtsp,
            tc.tile_pool(name="idx", bufs=1) as idxp,
            tc.tile_pool(name="msg", bufs=16) as msgp,
            tc.tile_pool(name="sel", bufs=12) as selp,
            tc.tile_pool(name="cmp", bufs=4) as cmpp,
            tc.tile_pool(name="agg", bufs=3, space="PSUM") as aggp,
            tc.tile_pool(name="mmp", bufs=2, space="PSUM") as mmpp,
        ):
            w = {}
            for name in ("W1", "W2", "Wu_t", "Wu_b", "Wi_t", "Wi_b"):
                w[name] = wtsp.tile([128, 128], F32, tag=name, name=f"w_{name}")
                nc.sync.dma_start(w[name][:], wn[name][:])
            for name in ("b1", "b2", "bu", "bi"):
                w[name] = wtsp.tile([128, 1], F32, tag=name, name=f"w_{name}")
                nc.sync.dma_start(w[name][:], wn[name][:])
            t_iota = wtsp.tile([128, TW], F32, tag="iota")
            nc.sync.dma_start(t_iota[:], iota[:])

            t_gs, t_gd = {}, {}
            for key, gs, gd, ep in (("u", gsu, gdu, epu), ("i", gsi, gdi, epi)):
                t_gs[key] = idxp.tile([128, ep // 128], I32, tag=f"gs{key}", name=f"t_gs_{key}")
                t_gd[key] = idxp.tile([128, ep // 128], F32, tag=f"gd{key}", name=f"t_gd_{key}")
                nc.sync.dma_start(t_gs[key][:], gs[:])
                nc.sync.dma_start(t_gd[key][:], gd[:])

            def direction(key, table, nch, feaT, W1n, b1n, Wtn, Wbn, btn,
                          nT_out, hT_out):
                gs, gd = t_gs[key], t_gd[key]
                col = 0
                with nc.named_scope(f"dir_{key}"):
                    for t in range(NT):
                        n = nch[t]
                        psA = aggp.tile([128, TW], F32, tag="psA")
                        # per 128-edge chunk: indirect gather (one row per
                        # partition; HW only honors [128,1] offset APs),
                        # one-hot S, accumulate msgs^T @ S into psum
                        for c in range(n):
                            mt = msgp.tile([128, 128], BF16, tag="mt")
                            nc.gpsimd.indirect_dma_start(
                                out=mt[:], out_offset=None,
                                in_=table[:],
                                in_offset=bass.IndirectOffsetOnAxis(
                                    ap=gs[:, col + c:col + c + 1], axis=0),
                            )
                            st = selp.tile([128, TW], BF16, tag="st")
                            nc.vector.tensor_scalar(
                                st[:], t_iota[:],
                                gd[:, col + c:col + c + 1], None,
                                mybir.AluOpType.is_equal)
                            nc.tensor.matmul(
                                psA[:], mt[:], st[:],
                                start=(c == 0), stop=(c == n - 1))
                        col += n

                        # dense tail for this 256-dst tile
                        j0 = t * TW
                        aggT = cmpp.tile([128, TW], F32, tag="aggT")
                        nc.vector.tensor_copy(aggT[:], psA[:])
                        pn = mmpp.tile([128, TW], F32, tag="pn")
                        nc.tensor.matmul(pn[:], w[W1n][:], aggT[:],
                                         start=True, stop=True)
                        # leaky_relu(pn + b1) = max(y, alpha*y)
                        yt = cmpp.tile([128, TW], F32, tag="yt")
                        nc.vector.tensor_scalar_add(yt[:], pn[:], w[b1n][:])
                        zt = cmpp.tile([128, TW], F32, tag="zt")
                        nc.vector.tensor_scalar_mul(zt[:], yt[:], ALPHA)
                        nT = cmpp.tile([128, TW], F32, tag="nT")
                        nc.vector.tensor_tensor(nT[:], yt[:], zt[:],
                                                mybir.AluOpType.max)
                        nc.sync.dma_start(nT_out[:, j0:j0 + TW], nT[:])

                        ft = cmpp.tile([128, TW], F32, tag="ft")
                        fdt = min(TW, max(0, SH - j0))
                        if fdt < TW:
                            nc.vector.memset(ft[:], 0.0)
                        if fdt > 0:
                            nc.sync.dma_start(ft[:, :fdt], feaT[:, j0:j0 + fdt])
                        ph = mmpp.tile([128, TW], F32, tag="ph")
                        nc.tensor.matmul(ph[:], w[Wtn][:], ft[:],
                                         start=True, stop=False)
                        nc.tensor.matmul(ph[:], w[Wbn][:], nT[:],
                                         start=False, stop=True)
                        hT = cmpp.tile([128, TW], F32, tag="hT")
                        nc.scalar.activation(
                            hT[:], ph[:], mybir.ActivationFunctionType.Relu,
                            bias=w[btn][:], scale=1.0)
                        nc.sync.dma_start(hT_out[:, j0:j0 + TW], hT[:])

            direction("u", vtab, nch_u, ufeaT, "W1", "b1", "Wu_t", "Wu_b",
                      "bu", unT, uhT)
            direction("i", utab, nch_i, vfeaT, "W2", "b2", "Wi_t", "Wi_b",
                      "bi", inT, ihT)
    return nc


def _run(ufea, vfea, uv_rows, uv_cols, trace=False):
    su_l, du_l, nch_u = _prep_direction(uv_rows, uv_cols)
    si_l, di_l, nch_i = _prep_direction(uv_cols, uv_rows)
    epu = 128 * sum(nch_u)
    epi = 128 * sum(nch_i)

    nc = bacc.Bacc("TRN2", target_bir_lowering=False, debug=False,
                   dynamic_dma_scratch_size=2**16)
    _build(nc, nch_u, nch_i, epu, epi)
    nc.compile()

    import ml_dtypes
    common = {
        "vtab": vfea.astype(ml_dtypes.bfloat16),
        "utab": ufea.astype(ml_dtypes.bfloat16),
        "iota": np.tile(np.arange(TW, dtype=np.float32), (128, 1)),
    }
    in_maps = []
    for c in range(NCORES):
        m = dict(common)
        m["ufeaT"] = np.ascontiguousarray(ufea[c * SH:(c + 1) * SH].T)
        m["vfeaT"] = np.ascontiguousarray(vfea[c * SH:(c + 1) * SH].T)
        m["gsu"], m["gdu"] = su_l[c], du_l[c]
        m["gsi"], m["gdi"] = si_l[c], di_l[c]
        in_maps.append(m)
    return nc, in_maps


def kernel(ufea, vfea, uv_rows, uv_cols, uv_vals,
           W1, b1, W2, b2, Wu, bu, Wi, bi) -> np.ndarray:
    ufea = np.ascontiguousarray(np.asarray(ufea, np.float32))
    vfea = np.ascontiguousarray(np.asarray(vfea, np.float32))
    uv_rows = np.asarray(uv_rows, np.int32)
    uv_cols = np.asarray(uv_cols, np.int32)

    nc, in_maps = _run(ufea, vfea, uv_rows, uv_cols)
    Wu = np.asarray(Wu, np.float32)
    Wi = np.asarray(Wi, np.float32)
    for m in in_maps:
        m.update({
            "W1": np.asarray(W1, np.float32), "W2": np.asarray(W2, np.float32),
            "Wu_t": np.ascontiguousarray(Wu[:128]),
            "Wu_b": np.ascontiguousarray(Wu[128:]),
            "Wi_t": np.ascontiguousarray(Wi[:128]),
            "Wi_b": np.ascontiguousarray(Wi[128:]),
            "b1": np.asarray(b1, np.float32).reshape(128, 1),
            "b2": np.asarray(b2, np.float32).reshape(128, 1),
            "bu": np.asarray(bu, np.float32).reshape(128, 1),
            "bi": np.asarray(bi, np.float32).reshape(128, 1),
        })

    res = bass_utils.run_bass_kernel_spmd(nc, in_maps, list(range(NCORES)),
                                          trace=False)

    out = np.empty((6, N_NODES, D), np.float32)
    for c in range(NCORES):
        r = res.results[c]
        sl = slice(c * SH, (c + 1) * SH)
        out[0][sl] = r["uhT"][:, :SH].T
        out[1][sl] = r["unT"][:, :SH].T
        out[3][sl] = r["ihT"][:, :SH].T
        out[4][sl] = r["inT"][:, :SH].T
    out[2] = ufea
    out[5] = vfea
    return out



# revision 6
# speedup vs baseline: 1.7764x; 1.7764x over previous
"""DGCNLayer (layer%2==0 branch) on 8 Trainium2 NeuronCores via Bass.

Math (per reference, with uv_vals == 1 per the problem spec and using
linearity to pull the dense GEMM past the segment-sum):
  User_n = leaky_relu(segsum_{rows}(vfea[cols]) @ W1 + b1, 0.1)
  Item_n = leaky_relu(segsum_{cols}(ufea[rows]) @ W2 + b2, 0.1)
  User_h = relu(concat([ufea, User_n]) @ Wu + bu)
  Item_h = relu(concat([vfea, Item_n]) @ Wi + bi)
  return stack([User_h, User_n, ufea, Item_h, Item_n, vfea])

Distribution: destination nodes are sharded 12500/core across the 8
cores; the gather tables (full vfea/ufea, bf16) are replicated into
every core's HBM so no collectives are needed.

Per core and direction, edges are grouped by destination tile (TW dsts)
and by source bin (int16-addressable 32768-row ranges of the table, a
dma_gather constraint), each (tile, bin) run padded to a multiple of
128 with index-0 slots mapped to one-hot column -1.  A handful of
large dma_gather calls per tile-group fetch all source rows (one edge
per partition, 128-edge chunks along the free dim) — this replaces the
per-128-edge indirect DMAs of the previous version whose ~1us SWDGE
descriptor-generation cost on GpSimd dominated the runtime.

Aggregation: for each 128-edge chunk, DVE builds a one-hot S
[128, TW] via is_equal(iota, dst), and TensorE accumulates
psum[f, d] += msgs^T @ S.  The dense tail (W1 matmul + LeakyReLU via
ACT, union matmuls + ReLU via ACT) is software-pipelined two tiles
behind the aggregation so PE never stalls on ACT.  Outputs are written
feature-major bf16 [128, 12544] and the host reassembles the stack.
"""
import sys
sys.path.insert(0, "/opt/trn_rl_repo")
import numpy as np

from concourse import bass, bacc, mybir
from concourse import bass_utils
from concourse.tile import TileContext

F32 = mybir.dt.float32
BF16 = mybir.dt.bfloat16
I16 = mybir.dt.int16

NCORES = 8
N_NODES = 100000
SH = N_NODES // NCORES      # 12500 destinations per core
D = 128
TW = 256                    # dst-tile width (psum half-bank)
AGG_ROWS = 12544            # 12500 padded to x256
NT = AGG_ROWS // TW         # dst tiles
GT = 5                      # tiles per gather group
BINSZ = 32768               # int16-addressable source bin
NBINS = (N_NODES + BINSZ - 1) // BINSZ
ALPHA = 0.1


def _prep_direction(dst_all: np.ndarray, src_all: np.ndarray):
    """Static layout + per-core tables for one direction.

    Edges are bucketed per core by (group, bin, tile) where
    group = tile // GT, and each (tile, bin) cell is padded to a
    cross-core-common multiple of 128 slots (pad: src index 0, dst -1).

    Returns (meta, percore) where
      meta: dict with
        pb[t][b]        padded slot count per (tile, bin)
        groups          list of (t0, t1)
        gcalls[g][b]    (idx_off_slots, num_idxs, chunk_off_in_group)
        tile_runs[t]    list of (group_local_chunk, nchunks, global_chunk)
        gchunks[g]      chunks in group g
        ch_max          max chunks over groups
        total_slots     total slot count
      percore[c]: dict with idx16 [128, slots//16] i16,
                  gd [128, slots//128] f32 (host converts to bf16)
    """
    ngroups = (NT + GT - 1) // GT
    groups = [(g * GT, min(NT, (g + 1) * GT)) for g in range(ngroups)]

    # per-core sorted edge arrays + per-(t,b) counts
    cores = []
    cnts = np.zeros((NCORES, NT, NBINS), np.int64)
    for c in range(NCORES):
        m = (dst_all >= c * SH) & (dst_all < (c + 1) * SH)
        d = (dst_all[m] - c * SH).astype(np.int64)
        s = src_all[m].astype(np.int64)
        t = d // TW
        b = s // BINSZ
        g = t // GT
        key = ((g * NBINS + b) * GT) + (t % GT)   # (group, bin, tile) order
        o = np.argsort(key, kind="stable")
        d, s, key = d[o], s[o], key[o]
        np.add.at(cnts[c], (t[o], b[o]), 1)
        cores.append((d, s, key))

    pb = 128 * np.maximum(1, -(-cnts.max(axis=0) // 128)) * (
        cnts.max(axis=0) > 0)
    # guarantee every tile has at least one chunk
    for t in range(NT):
        if pb[t].sum() == 0:
            pb[t][0] = 128
    pb = pb.astype(np.int64)

    # static layout in (group, bin, tile) order
    cell_off = np.zeros((NT, NBINS), np.int64)    # slot offset of each cell
    gcalls = []
    tile_runs = [[] for _ in range(NT)]
    gchunks = []
    off = 0
    for (t0, t1) in groups:
        calls = []
        goff = off
        for b in range(NBINS):
            n = int(pb[t0:t1, b].sum())
            calls.append((off, n, (off - goff) // 128))
            for t in range(t0, t1):
                cell_off[t][b] = off
                k = int(pb[t][b]) // 128
                if k:
                    tile_runs[t].append(
                        ((off - goff) // 128, k, off // 128))
                off += int(pb[t][b])
        gcalls.append(calls)
        gchunks.append((off - goff) // 128)
    total_slots = off
    meta = dict(pb=pb, groups=groups, gcalls=gcalls, tile_runs=tile_runs,
                gchunks=gchunks, ch_max=max(gchunks), total_slots=total_slots)

    # per-core tables
    percore = []
    for c in range(NCORES):
        d, s, key = cores[c]
        # rank within cell
        first = np.zeros(ngroups * NBINS * GT, np.int64)
        np.add.at(first, key, 1)
        cstart = np.concatenate([[0], np.cumsum(first)[:-1]])
        rank = np.arange(len(key)) - cstart[key]
        t = np.minimum(d // TW, NT - 1)
        b = s // BINSZ
        pos = cell_off[t, b] + rank
        idx = np.zeros(total_slots, np.int16)
        gd = np.full(total_slots, -1.0, np.float32)
        idx[pos] = (s - b * BINSZ).astype(np.int16)
        gd[pos] = (d - t * TW).astype(np.float32)
        idx16 = np.tile(idx.reshape(-1, 16).T, (8, 1)).copy()  # [128, slots/16]
        gdw = gd.reshape(-1, 128).T.copy()                     # [128, slots/128]
        percore.append(dict(idx16=idx16, gd=gdw))
    return meta, percore


def _build(nc: bass.Bass, mu: dict, mi: dict):
    vtab = nc.dram_tensor("vtab", [N_NODES, D], BF16, kind="ExternalInput")
    utab = nc.dram_tensor("utab", [N_NODES, D], BF16, kind="ExternalInput")
    ufeaT = nc.dram_tensor("ufeaT", [128, AGG_ROWS], BF16, kind="ExternalInput")
    vfeaT = nc.dram_tensor("vfeaT", [128, AGG_ROWS], BF16, kind="ExternalInput")
    gidx_u = nc.dram_tensor("gidx_u", [128, mu["total_slots"] // 16], I16,
                            kind="ExternalInput")
    gidx_i = nc.dram_tensor("gidx_i", [128, mi["total_slots"] // 16], I16,
                            kind="ExternalInput")
    gd_u = nc.dram_tensor("gd_u", [128, mu["total_slots"] // 128], F32,
                          kind="ExternalInput")
    gd_i = nc.dram_tensor("gd_i", [128, mi["total_slots"] // 128], F32,
                          kind="ExternalInput")
    iota = nc.dram_tensor("iota", [128, TW], F32, kind="ExternalInput")
    wn = {}
    for w in ("W1", "W2", "Wu_t", "Wu_b", "Wi_t", "Wi_b"):
        wn[w] = nc.dram_tensor(w, [128, 128], BF16, kind="ExternalInput")
    for b in ("b1", "b2", "bu", "bi"):
        wn[b] = nc.dram_tensor(b, [128, 1], F32, kind="ExternalInput")

    unT = nc.dram_tensor("unT", [128, AGG_ROWS], BF16, kind="ExternalOutput")
    uhT = nc.dram_tensor("uhT", [128, AGG_ROWS], BF16, kind="ExternalOutput")
    inT = nc.dram_tensor("inT", [128, AGG_ROWS], BF16, kind="ExternalOutput")
    ihT = nc.dram_tensor("ihT", [128, AGG_ROWS], BF16, kind="ExternalOutput")

    ch_max = max(mu["ch_max"], mi["ch_max"])

    with TileContext(nc) as tc:
        with (
            tc.tile_pool(name="wts", bufs=1) as wtsp,
            tc.tile_pool(name="fea", bufs=1) as feap,
            tc.tile_pool(name="idx", bufs=1) as idxp,
            tc.tile_pool(name="msg", bufs=2) as msgp,
            tc.tile_pool(name="sel", bufs=10) as selp,
            tc.tile_pool(name="cmp", bufs=3) as cmpp,
            tc.tile_pool(name="ntp", bufs=3) as ntpp,
            tc.tile_pool(name="htp", bufs=2) as htpp,
            tc.tile_pool(name="agg", bufs=3, space="PSUM") as aggp,
            tc.tile_pool(name="pnp", bufs=2, space="PSUM") as pnpp,
            tc.tile_pool(name="php", bufs=2, space="PSUM") as phpp,
        ):
            w = {}
            for name in ("W1", "W2", "Wu_t", "Wu_b", "Wi_t", "Wi_b"):
                w[name] = wtsp.tile([128, 128], BF16, tag=name, name=f"w_{name}")
                nc.sync.dma_start(w[name][:], wn[name][:])
            for name in ("b1", "b2", "bu", "bi"):
                w[name] = wtsp.tile([128, 1], F32, tag=name, name=f"w_{name}")
                nc.sync.dma_start(w[name][:], wn[name][:])
            t_iota = wtsp.tile([128, TW], F32, tag="iota")
            nc.sync.dma_start(t_iota[:], iota[:])
            t_alpha = wtsp.tile([128, 1], F32, tag="alpha")
            nc.vector.memset(t_alpha[:], ALPHA)

            t_ft, t_gi, t_gd = {}, {}, {}
            for key, feat, gi, gdt, m in (("u", ufeaT, gidx_u, gd_u, mu),
                                          ("i", vfeaT, gidx_i, gd_i, mi)):
                t_ft[key] = feap.tile([128, AGG_ROWS], BF16, tag=f"ft{key}",
                                      name=f"t_ft_{key}")
                nc.sync.dma_start(t_ft[key][:], feat[:])
                t_gi[key] = idxp.tile([128, m["total_slots"] // 16], I16,
                                      tag=f"gi{key}", name=f"t_gi_{key}")
                nc.sync.dma_start(t_gi[key][:], gi[:])
                t_gd[key] = idxp.tile([128, m["total_slots"] // 128], F32,
                                      tag=f"gd{key}", name=f"t_gd_{key}")
                nc.sync.dma_start(t_gd[key][:], gdt[:])

            def direction(key, table, m, W1n, b1n, Wtn, Wbn, btn,
                          nT_out, hT_out):
                gi, gd, ft = t_gi[key], t_gd[key], t_ft[key]
                pend = []  # software pipeline: [(t, psA), ...]

                def stageA(ent):
                    t, psA = ent
                    j0 = t * TW
                    aggT = cmpp.tile([128, TW], BF16, tag="aggT")
                    nc.vector.tensor_copy(aggT[:], psA[:])
                    pn = pnpp.tile([128, TW], F32, tag="pn")
                    nc.tensor.matmul(pn[:], w[W1n][:], aggT[:],
                                     start=True, stop=True)
                    nT = ntpp.tile([128, TW], BF16, tag="nT")
                    nc.scalar.activation(
                        nT[:], pn[:], mybir.ActivationFunctionType.Prelu,
                        bias=w[b1n][:], scale=1.0, alpha=t_alpha[:])
                    nc.sync.dma_start(nT_out[:, j0:j0 + TW], nT[:])
                    ent.append(nT)

                def stageB(ent):
                    t, psA, nT = ent
                    j0 = t * TW
                    ph = phpp.tile([128, TW], F32, tag="ph")
                    nc.tensor.matmul(ph[:], w[Wtn][:], ft[:, j0:j0 + TW],
                                     start=True, stop=False)
                    nc.tensor.matmul(ph[:], w[Wbn][:], nT[:],
                                     start=False, stop=True)
                    hT = htpp.tile([128, TW], BF16, tag="hT")
                    nc.scalar.activation(
                        hT[:], ph[:], mybir.ActivationFunctionType.Relu,
                        bias=w[btn][:], scale=1.0)
                    nc.sync.dma_start(hT_out[:, j0:j0 + TW], hT[:])

                with nc.named_scope(f"dir_{key}"):
                    for g, (t0, t1) in enumerate(m["groups"]):
                        mt = msgp.tile([128, ch_max, 128], BF16, tag="mt")
                        for b in range(NBINS):
                            ioff, nidx, coff = m["gcalls"][g][b]
                            if nidx == 0:
                                continue
                            bhi = min(N_NODES, (b + 1) * BINSZ)
                            nc.gpsimd.dma_gather(
                                out_ap=mt[:, coff:coff + nidx // 128, :],
                                in_ap=table[b * BINSZ:bhi, :],
                                idxs_ap=gi[:, ioff // 16:(ioff + nidx) // 16],
                                num_idxs=nidx,
                                num_idxs_reg=nidx,
                                elem_size=D,
                                single_packet=False,
                                queue_num=b,
                            )
                        for t in range(t0, t1):
                            runs = m["tile_runs"][t]
                            K = sum(r[1] for r in runs)
                            psA = aggp.tile([128, TW], F32, tag="psA")
                            j = 0
                            for (lc, k, gc) in runs:
                                for q in range(k):
                                    st = selp.tile([128, TW], BF16, tag="st")
                                    nc.vector.tensor_scalar(
                                        st[:], t_iota[:],
                                        gd[:, gc + q:gc + q + 1], None,
                                        mybir.AluOpType.is_equal)
                                    nc.tensor.matmul(
                                        psA[:], mt[:, lc + q, :], st[:],
                                        start=(j == 0), stop=(j == K - 1))
                                    j += 1
                            pend.append([t, psA])
                            if len(pend) >= 2:
                                stageA(pend[-2])
                            if len(pend) >= 3:
                                stageB(pend.pop(0))
                    # drain
                    if pend:
                        stageA(pend[-1])
                    for ent in pend:
                        stageB(ent)

            direction("u", vtab, mu, "W1", "b1", "Wu_t", "Wu_b", "bu",
                      unT, uhT)
            direction("i", utab, mi, "W2", "b2", "Wi_t", "Wi_b", "bi",
                      inT, ihT)
    return nc


def _run(ufea, vfea, uv_rows, uv_cols):
    mu, pcu = _prep_direction(uv_rows, uv_cols)
    mi, pci = _prep_direction(uv_cols, uv_rows)

    nc = bacc.Bacc("TRN2", target_bir_lowering=False, debug=False,
                   dynamic_dma_scratch_size=2**16, num_swdge_queues=4)
    _build(nc, mu, mi)
    nc.compile()

    import ml_dtypes
    bf = ml_dtypes.bfloat16
    common = {
        "vtab": vfea.astype(bf),
        "utab": ufea.astype(bf),
        "iota": np.tile(np.arange(TW, dtype=np.float32), (128, 1)),
    }
    in_maps = []
    for c in range(NCORES):
        m = dict(common)
        fu = np.zeros((128, AGG_ROWS), np.float32)
        fv = np.zeros((128, AGG_ROWS), np.float32)
        fu[:, :SH] = ufea[c * SH:(c + 1) * SH].T
        fv[:, :SH] = vfea[c * SH:(c + 1) * SH].T
        m["ufeaT"] = fu.astype(bf)
        m["vfeaT"] = fv.astype(bf)
        m["gidx_u"] = pcu[c]["idx16"]
        m["gd_u"] = pcu[c]["gd"]
        m["gidx_i"] = pci[c]["idx16"]
        m["gd_i"] = pci[c]["gd"]
        in_maps.append(m)
    return nc, in_maps


def kernel(ufea, vfea, uv_rows, uv_cols, uv_vals,
           W1, b1, W2, b2, Wu, bu, Wi, bi) -> np.ndarray:
    import ml_dtypes
    bf = ml_dtypes.bfloat16
    ufea = np.ascontiguousarray(np.asarray(ufea, np.float32))
    vfea = np.ascontiguousarray(np.asarray(vfea, np.float32))
    uv_rows = np.asarray(uv_rows, np.int32)
    uv_cols = np.asarray(uv_cols, np.int32)

    nc, in_maps = _run(ufea, vfea, uv_rows, uv_cols)
    Wu = np.asarray(Wu, np.float32)
    Wi = np.asarray(Wi, np.float32)
    for m in in_maps:
        m.update({
            "W1": np.asarray(W1, np.float32).astype(bf),
            "W2": np.asarray(W2, np.float32).astype(bf),
            "Wu_t": np.ascontiguousarray(Wu[:128]).astype(bf),
            "Wu_b": np.ascontiguousarray(Wu[128:]).astype(bf),
            "Wi_t": np.ascontiguousarray(Wi[:128]).astype(bf),
            "Wi_b": np.ascontiguousarray(Wi[128:]).astype(bf),
            "b1": np.asarray(b1, np.float32).reshape(128, 1),
            "b2": np.asarray(b2, np.float32).reshape(128, 1),
            "bu": np.asarray(bu, np.float32).reshape(128, 1),
            "bi": np.asarray(bi, np.float32).reshape(128, 1),
        })

    res = bass_utils.run_bass_kernel_spmd(nc, in_maps, list(range(NCORES)),
                                          trace=False)

    out = np.empty((6, N_NODES, D), np.float32)
    for c in range(NCORES):
        r = res.results[c]
        sl = slice(c * SH, (c + 1) * SH)
        out[0][sl] = r["uhT"][:, :SH].T.astype(np.float32)
        out[1][sl] = r["unT"][:, :SH].T.astype(np.float32)
        out[3][sl] = r["ihT"][:, :SH].T.astype(np.float32)
        out[4][sl] = r["inT"][:, :SH].T.astype(np.float32)
    out[2] = ufea
    out[5] = vfea
    return out


# revision 7
# speedup vs baseline: 2.1509x; 1.2109x over previous
"""DGCNLayer (layer%2==0 branch) on 8 Trainium2 NeuronCores via Bass.

Math (per reference, with uv_vals == 1 per the problem spec and using
linearity to pull the dense GEMM past the segment-sum):
  User_n = leaky_relu(segsum_{rows}(vfea[cols]) @ W1 + b1, 0.1)
  Item_n = leaky_relu(segsum_{cols}(ufea[rows]) @ W2 + b2, 0.1)
  User_h = relu(concat([ufea, User_n]) @ Wu + bu)
  Item_h = relu(concat([vfea, Item_n]) @ Wi + bi)
  return stack([User_h, User_n, ufea, Item_h, Item_n, vfea])

Distribution: destination nodes are sharded 12500/core across the 8
cores; the gather tables (full vfea/ufea, bf16) are replicated into
every core's HBM so no collectives are needed.

Per core and direction, edges are grouped by destination tile (TW dsts)
and by source bin (int16-addressable 32768-row ranges of the table, a
dma_gather constraint), each (tile, bin) run padded to a multiple of
128 with index-0 slots mapped to one-hot column -1.  A handful of
large dma_gather calls per tile-group fetch all source rows (one edge
per partition, 128-edge chunks along the free dim) — this replaces the
per-128-edge indirect DMAs of the previous version whose ~1us SWDGE
descriptor-generation cost on GpSimd dominated the runtime.

Aggregation: for each 128-edge chunk, DVE builds a one-hot S
[128, TW] via is_equal(iota, dst), and TensorE accumulates
psum[f, d] += msgs^T @ S.  The dense tail (W1 matmul + LeakyReLU via
ACT, union matmuls + ReLU via ACT) is software-pipelined two tiles
behind the aggregation so PE never stalls on ACT.  Outputs are written
feature-major bf16 [128, 12544] and the host reassembles the stack.
"""
import sys
sys.path.insert(0, "/opt/trn_rl_repo")
import numpy as np

from concourse import bass, bacc, mybir
from concourse import bass_utils
from concourse.tile import TileContext

F32 = mybir.dt.float32
BF16 = mybir.dt.bfloat16
FP8 = mybir.dt.float8e4
I16 = mybir.dt.int16

NCORES = 8
N_NODES = 100000
SH = N_NODES // NCORES      # 12500 destinations per core
D = 128
TW = 256                    # dst-tile width (psum half-bank)
AGG_ROWS = 12544            # 12500 padded to x256
NT = AGG_ROWS // TW         # dst tiles
GT = 5                      # tiles per gather group
BINSZ = 32768               # int16-addressable source bin
NBINS = (N_NODES + BINSZ - 1) // BINSZ
ALPHA = 0.1


def _prep_direction(dst_all: np.ndarray, src_all: np.ndarray):
    """Static layout + per-core tables for one direction.

    Edges are bucketed per core by (group, bin, tile) where
    group = tile // GT, and each (tile, bin) cell is padded to a
    cross-core-common multiple of 128 slots (pad: src index 0, dst -1).

    Returns (meta, percore) where
      meta: dict with
        pb[t][b]        padded slot count per (tile, bin)
        groups          list of (t0, t1)
        gcalls[g][b]    (idx_off_slots, num_idxs, chunk_off_in_group)
        tile_runs[t]    list of (group_local_chunk, nchunks, global_chunk)
        gchunks[g]      chunks in group g
        ch_max          max chunks over groups
        total_slots     total slot count
      percore[c]: dict with idx16 [128, slots//16] i16,
                  gd [128, slots//128] f32 (host converts to bf16)
    """
    ngroups = (NT + GT - 1) // GT
    groups = [(g * GT, min(NT, (g + 1) * GT)) for g in range(ngroups)]

    # per-core sorted edge arrays + per-(t,b) counts
    cores = []
    cnts = np.zeros((NCORES, NT, NBINS), np.int64)
    for c in range(NCORES):
        m = (dst_all >= c * SH) & (dst_all < (c + 1) * SH)
        d = (dst_all[m] - c * SH).astype(np.int64)
        s = src_all[m].astype(np.int64)
        t = d // TW
        b = s // BINSZ
        g = t // GT
        key = ((g * NBINS + b) * GT) + (t % GT)   # (group, bin, tile) order
        o = np.argsort(key, kind="stable")
        d, s, key = d[o], s[o], key[o]
        np.add.at(cnts[c], (t[o], b[o]), 1)
        cores.append((d, s, key))

    pb = 128 * np.maximum(1, -(-cnts.max(axis=0) // 128)) * (
        cnts.max(axis=0) > 0)
    # guarantee every tile has at least one chunk
    for t in range(NT):
        if pb[t].sum() == 0:
            pb[t][0] = 128
    pb = pb.astype(np.int64)

    # static layout in (group, bin, tile) order
    cell_off = np.zeros((NT, NBINS), np.int64)    # slot offset of each cell
    gcalls = []
    tile_runs = [[] for _ in range(NT)]
    gchunks = []
    off = 0
    for (t0, t1) in groups:
        calls = []
        goff = off
        for b in range(NBINS):
            n = int(pb[t0:t1, b].sum())
            calls.append((off, n, (off - goff) // 128))
            for t in range(t0, t1):
                cell_off[t][b] = off
                k = int(pb[t][b]) // 128
                if k:
                    tile_runs[t].append(
                        ((off - goff) // 128, k, off // 128))
                off += int(pb[t][b])
        gcalls.append(calls)
        gchunks.append((off - goff) // 128)
    total_slots = off
    meta = dict(pb=pb, groups=groups, gcalls=gcalls, tile_runs=tile_runs,
                gchunks=gchunks, ch_max=max(gchunks), total_slots=total_slots)

    # per-core tables
    percore = []
    for c in range(NCORES):
        d, s, key = cores[c]
        # rank within cell
        first = np.zeros(ngroups * NBINS * GT, np.int64)
        np.add.at(first, key, 1)
        cstart = np.concatenate([[0], np.cumsum(first)[:-1]])
        rank = np.arange(len(key)) - cstart[key]
        t = np.minimum(d // TW, NT - 1)
        b = s // BINSZ
        pos = cell_off[t, b] + rank
        idx = np.zeros(total_slots, np.int16)
        gd = np.full(total_slots, -1.0, np.float32)
        idx[pos] = (s - b * BINSZ).astype(np.int16)
        gd[pos] = (d - t * TW).astype(np.float32)
        idx16 = np.tile(idx.reshape(-1, 16).T, (8, 1)).copy()  # [128, slots/16]
        gdw = gd.reshape(-1, 128).T                            # [128, slots/128]
        percore.append(dict(idx16=idx16, gd=np.ascontiguousarray(gdw)))
    return meta, percore


def _build(nc: bass.Bass, mu: dict, mi: dict):
    vtab = nc.dram_tensor("vtab", [N_NODES, D], BF16, kind="ExternalInput")
    utab = nc.dram_tensor("utab", [N_NODES, D], BF16, kind="ExternalInput")
    ufeaT = nc.dram_tensor("ufeaT", [128, AGG_ROWS], BF16, kind="ExternalInput")
    vfeaT = nc.dram_tensor("vfeaT", [128, AGG_ROWS], BF16, kind="ExternalInput")
    gidx_u = nc.dram_tensor("gidx_u", [128, mu["total_slots"] // 16], I16,
                            kind="ExternalInput")
    gidx_i = nc.dram_tensor("gidx_i", [128, mi["total_slots"] // 16], I16,
                            kind="ExternalInput")
    s_u = nc.dram_tensor("s_u", [128, (mu["total_slots"] // 128) * TW], FP8,
                         kind="ExternalInput")
    s_i = nc.dram_tensor("s_i", [128, (mi["total_slots"] // 128) * TW], FP8,
                         kind="ExternalInput")
    wn = {}
    for w in ("W1", "W2", "Wu_t", "Wu_b", "Wi_t", "Wi_b"):
        wn[w] = nc.dram_tensor(w, [128, 128], BF16, kind="ExternalInput")
    for b in ("b1", "b2", "bu", "bi"):
        wn[b] = nc.dram_tensor(b, [128, 1], F32, kind="ExternalInput")

    unT = nc.dram_tensor("unT", [128, AGG_ROWS], BF16, kind="ExternalOutput")
    uhT = nc.dram_tensor("uhT", [128, AGG_ROWS], BF16, kind="ExternalOutput")
    inT = nc.dram_tensor("inT", [128, AGG_ROWS], BF16, kind="ExternalOutput")
    ihT = nc.dram_tensor("ihT", [128, AGG_ROWS], BF16, kind="ExternalOutput")

    ch_max = max(mu["ch_max"], mi["ch_max"])

    with TileContext(nc) as tc:
        with (
            tc.tile_pool(name="wts", bufs=1) as wtsp,
            tc.tile_pool(name="fea", bufs=1) as feap,
            tc.tile_pool(name="idx", bufs=1) as idxp,
            tc.tile_pool(name="msg", bufs=2) as msgp,
            tc.tile_pool(name="stp", bufs=2) as stpp,
            tc.tile_pool(name="cmp", bufs=3) as cmpp,
            tc.tile_pool(name="ntp", bufs=3) as ntpp,
            tc.tile_pool(name="htp", bufs=2) as htpp,
            tc.tile_pool(name="agg", bufs=3, space="PSUM") as aggp,
            tc.tile_pool(name="pnp", bufs=2, space="PSUM") as pnpp,
            tc.tile_pool(name="php", bufs=2, space="PSUM") as phpp,
        ):
            w = {}
            for name in ("W1", "W2", "Wu_t", "Wu_b", "Wi_t", "Wi_b"):
                w[name] = wtsp.tile([128, 128], BF16, tag=name, name=f"w_{name}")
                nc.sync.dma_start(w[name][:], wn[name][:])
            for name in ("b1", "b2", "bu", "bi"):
                w[name] = wtsp.tile([128, 1], F32, tag=name, name=f"w_{name}")
                nc.sync.dma_start(w[name][:], wn[name][:])
            t_alpha = wtsp.tile([128, 1], F32, tag="alpha")
            nc.vector.memset(t_alpha[:], ALPHA)

            t_ft, t_gi, s_dram = {}, {}, {"u": s_u, "i": s_i}
            for key, feat, gi, m in (("u", ufeaT, gidx_u, mu),
                                     ("i", vfeaT, gidx_i, mi)):
                t_ft[key] = feap.tile([128, AGG_ROWS], BF16, tag=f"ft{key}",
                                      name=f"t_ft_{key}")
                nc.sync.dma_start(t_ft[key][:], feat[:])
                t_gi[key] = idxp.tile([128, m["total_slots"] // 16], I16,
                                      tag=f"gi{key}", name=f"t_gi_{key}")
                nc.sync.dma_start(t_gi[key][:], gi[:])

            def direction(key, table, m, W1n, b1n, Wtn, Wbn, btn,
                          nT_out, hT_out):
                gi, sdram, ft = t_gi[key], s_dram[key], t_ft[key]
                pend = []  # software pipeline: [(t, psA), ...]

                def stageA(ent):
                    t, psA = ent
                    j0 = t * TW
                    aggT = cmpp.tile([128, TW], BF16, tag="aggT")
                    nc.vector.tensor_copy(aggT[:], psA[:])
                    pn = pnpp.tile([128, TW], F32, tag="pn")
                    nc.tensor.matmul(pn[:], w[W1n][:], aggT[:],
                                     start=True, stop=True)
                    nT = ntpp.tile([128, TW], BF16, tag="nT")
                    nc.scalar.activation(
                        nT[:], pn[:], mybir.ActivationFunctionType.Prelu,
                        bias=w[b1n][:], scale=1.0, alpha=t_alpha[:])
                    nc.sync.dma_start(nT_out[:, j0:j0 + TW], nT[:])
                    ent.append(nT)

                def stageB(ent):
                    t, psA, nT = ent
                    j0 = t * TW
                    ph = phpp.tile([128, TW], F32, tag="ph")
                    nc.tensor.matmul(ph[:], w[Wtn][:], ft[:, j0:j0 + TW],
                                     start=True, stop=False)
                    nc.tensor.matmul(ph[:], w[Wbn][:], nT[:],
                                     start=False, stop=True)
                    hT = htpp.tile([128, TW], BF16, tag="hT")
                    nc.scalar.activation(
                        hT[:], ph[:], mybir.ActivationFunctionType.Relu,
                        bias=w[btn][:], scale=1.0)
                    nc.sync.dma_start(hT_out[:, j0:j0 + TW], hT[:])

                with nc.named_scope(f"dir_{key}"):
                    gbase = 0
                    for g, (t0, t1) in enumerate(m["groups"]):
                        gch = m["gchunks"][g]
                        mt = msgp.tile([128, ch_max, 128], BF16, tag="mt")
                        stt = stpp.tile([128, ch_max, TW], FP8, tag="stt")
                        nc.sync.dma_start(
                            stt[:, :gch, :],
                            sdram[:, gbase * TW:(gbase + gch) * TW])
                        for b in range(NBINS):
                            ioff, nidx, coff = m["gcalls"][g][b]
                            if nidx == 0:
                                continue
                            bhi = min(N_NODES, (b + 1) * BINSZ)
                            nc.gpsimd.dma_gather(
                                out_ap=mt[:, coff:coff + nidx // 128, :],
                                in_ap=table[b * BINSZ:bhi, :],
                                idxs_ap=gi[:, ioff // 16:(ioff + nidx) // 16],
                                num_idxs=nidx,
                                num_idxs_reg=nidx,
                                elem_size=D,
                                single_packet=False,
                                queue_num=b,
                            )
                        for t in range(t0, t1):
                            runs = m["tile_runs"][t]
                            K = sum(r[1] for r in runs)
                            psA = aggp.tile([128, TW], F32, tag="psA")
                            j = 0
                            for (lc, k, gc) in runs:
                                for q in range(k):
                                    nc.tensor.matmul(
                                        psA[:], mt[:, lc + q, :],
                                        stt[:, lc + q, :],
                                        start=(j == 0), stop=(j == K - 1))
                                    j += 1
                            pend.append([t, psA])
                            if len(pend) >= 2:
                                stageA(pend[-2])
                            if len(pend) >= 3:
                                stageB(pend.pop(0))
                        gbase += gch
                    # drain
                    if pend:
                        stageA(pend[-1])
                    for ent in pend:
                        stageB(ent)

            direction("u", vtab, mu, "W1", "b1", "Wu_t", "Wu_b", "bu",
                      unT, uhT)
            direction("i", utab, mi, "W2", "b2", "Wi_t", "Wi_b", "bi",
                      inT, ihT)
    return nc


def _onehot_fp8(gdw: np.ndarray) -> np.ndarray:
    """[128, NCH] f32 dst columns -> [128, NCH*TW] fp8 one-hot rows."""
    import ml_dtypes
    nch = gdw.shape[1]
    s = (gdw[:, :, None] == np.arange(TW, dtype=np.float32)[None, None, :])
    return s.reshape(128, nch * TW).astype(ml_dtypes.float8_e4m3)


def _run(ufea, vfea, uv_rows, uv_cols):
    mu, pcu = _prep_direction(uv_rows, uv_cols)
    mi, pci = _prep_direction(uv_cols, uv_rows)

    nc = bacc.Bacc("TRN2", target_bir_lowering=False, debug=False,
                   dynamic_dma_scratch_size=2**16, num_swdge_queues=4)
    _build(nc, mu, mi)
    nc.compile()

    import ml_dtypes
    bf = ml_dtypes.bfloat16
    common = {
        "vtab": vfea.astype(bf),
        "utab": ufea.astype(bf),
    }
    in_maps = []
    for c in range(NCORES):
        m = dict(common)
        fu = np.zeros((128, AGG_ROWS), np.float32)
        fv = np.zeros((128, AGG_ROWS), np.float32)
        fu[:, :SH] = ufea[c * SH:(c + 1) * SH].T
        fv[:, :SH] = vfea[c * SH:(c + 1) * SH].T
        m["ufeaT"] = fu.astype(bf)
        m["vfeaT"] = fv.astype(bf)
        m["gidx_u"] = pcu[c]["idx16"]
        m["s_u"] = _onehot_fp8(pcu[c]["gd"])
        m["gidx_i"] = pci[c]["idx16"]
        m["s_i"] = _onehot_fp8(pci[c]["gd"])
        in_maps.append(m)
    return nc, in_maps


def kernel(ufea, vfea, uv_rows, uv_cols, uv_vals,
           W1, b1, W2, b2, Wu, bu, Wi, bi) -> np.ndarray:
    import ml_dtypes
    bf = ml_dtypes.bfloat16
    ufea = np.ascontiguousarray(np.asarray(ufea, np.float32))
    vfea = np.ascontiguousarray(np.asarray(vfea, np.float32))
    uv_rows = np.asarray(uv_rows, np.int32)
    uv_cols = np.asarray(uv_cols, np.int32)

    nc, in_maps = _run(ufea, vfea, uv_rows, uv_cols)
    Wu = np.asarray(Wu, np.float32)
    Wi = np.asarray(Wi, np.float32)
    for m in in_maps:
        m.update({
            "W1": np.asarray(W1, np.float32).astype(bf),
            "W2": np.asarray(W2, np.float32).astype(bf),
            "Wu_t": np.ascontiguousarray(Wu[:128]).astype(bf),
            "Wu_b": np.ascontiguousarray(Wu[128:]).astype(bf),
            "Wi_t": np.ascontiguousarray(Wi[:128]).astype(bf),
            "Wi_b": np.ascontiguousarray(Wi[128:]).astype(bf),
            "b1": np.asarray(b1, np.float32).reshape(128, 1),
            "b2": np.asarray(b2, np.float32).reshape(128, 1),
            "bu": np.asarray(bu, np.float32).reshape(128, 1),
            "bi": np.asarray(bi, np.float32).reshape(128, 1),
        })

    res = bass_utils.run_bass_kernel_spmd(nc, in_maps, list(range(NCORES)),
                                          trace=False)

    out = np.empty((6, N_NODES, D), np.float32)
    for c in range(NCORES):
        r = res.results[c]
        sl = slice(c * SH, (c + 1) * SH)
        out[0][sl] = r["uhT"][:, :SH].T.astype(np.float32)
        out[1][sl] = r["unT"][:, :SH].T.astype(np.float32)
        out[3][sl] = r["ihT"][:, :SH].T.astype(np.float32)
        out[4][sl] = r["inT"][:, :SH].T.astype(np.float32)
    out[2] = ufea
    out[5] = vfea
    return out


# revision 10
# speedup vs baseline: 3.6553x; 1.6994x over previous
"""DGCNLayer (layer%2==0 branch) on 8 Trainium2 NeuronCores via Bass.

Math (per reference, with uv_vals == 1 per the problem spec and using
linearity to pull the dense GEMM past the segment-sum):
  User_n = leaky_relu(segsum_{rows}(vfea[cols]) @ W1 + b1, 0.1)
  Item_n = leaky_relu(segsum_{cols}(ufea[rows]) @ W2 + b2, 0.1)
  User_h = relu(concat([ufea, User_n]) @ Wu + bu)
  Item_h = relu(concat([vfea, Item_n]) @ Wi + bi)
  return stack([User_h, User_n, ufea, Item_h, Item_n, vfea])

Distribution: destination nodes are sharded 12500/core across the 8
cores; the gather tables (full vfea/ufea, bf16) are replicated into
every core's HBM so no collectives are needed.

Per core and direction, edges are grouped by (gather group of GT dst
tiles, source bin, dst tile).  Source bins are the int16-addressable
32768-row windows of the table (dma_gather constraint).  Cell sizes are
the cross-core max edge count (exact, not chunk-padded); large
dma_gather calls (one per (group, bin), spread over the 4 SWDGE queues)
fetch the source rows, one edge slot per partition, 128-slot chunks
along the free dim.  Chunks straddling a tile boundary are matmul'd by
both tiles.

Aggregation: the one-hot matrices S [128, TW] per chunk-matmul are
precomputed on the host in fp8 (exact 0/1, masked to the owning tile;
pad slots all-zero) and streamed from HBM by HWDGE — the DVE-built
is_equal one-hots of earlier versions suffered a ~2.5x slowdown from
SWDGE descriptor-ring interference.  TensorE accumulates
psum[f, d] += msgs^T @ S per chunk.  The dense tail (W1 matmul + leaky
via ACT Prelu, union matmuls + ReLU via ACT) is software-pipelined two
tiles behind the aggregation, and the two directions are interleaved at
group granularity to keep the gather queues saturated.  Outputs are
written feature-major bf16 [128, 12544] and the host reassembles.
"""
import sys
sys.path.insert(0, "/opt/trn_rl_repo")
import numpy as np

from concourse import bass, bacc, mybir
from concourse import bass_utils
from concourse.tile import TileContext

F32 = mybir.dt.float32
BF16 = mybir.dt.bfloat16
FP8 = mybir.dt.float8e4
I16 = mybir.dt.int16

NCORES = 8
N_NODES = 100000
SH = N_NODES // NCORES      # 12500 destinations per core
D = 128
TW = 256                    # dst-tile width (psum half-bank)
AGG_ROWS = 12544            # 12500 padded to x256
NT = AGG_ROWS // TW         # dst tiles
GT = 4                      # tiles per gather group
BINSZ = 32768               # int16-addressable source bin
NBINS = (N_NODES + BINSZ - 1) // BINSZ
ALPHA = 0.1


def _prep_direction(dst_all: np.ndarray, src_all: np.ndarray):
    """Static layout + per-core tables for one direction.

    Cells (group, bin, tile) are packed back-to-back with size
    pb[t][b] = max_c count — no chunk padding; only each (group, bin)
    gather call is rounded up to a multiple of 128 slots (pad idx 0,
    all-zero S rows).

    meta (static across cores):
      groups            [(t0, t1)]
      gcalls[g][b]      (idx_off_slots, num_idxs)
      gslot0[g]         slot offset of group g
      gchunks[g]        gather chunks in group g
      mms[g]            [(tile, local_chunk, s_col)] chunk-matmuls
      nmm_max, ch_max, total_slots, total_mm
    percore[c]:
      idx16 [128, slots//16] i16
      sfp8  [128, total_mm * TW] fp8 one-hot stream
    """
    ngroups = (NT + GT - 1) // GT
    groups = [(g * GT, min(NT, (g + 1) * GT)) for g in range(ngroups)]

    # per-core edge arrays sorted by (group, bin, tile); per-cell counts
    cores = []
    cnts = np.zeros((NCORES, NT, NBINS), np.int64)
    for c in range(NCORES):
        m = (dst_all >= c * SH) & (dst_all < (c + 1) * SH)
        d = (dst_all[m] - c * SH).astype(np.int64)
        s = src_all[m].astype(np.int64)
        t = d // TW
        b = s // BINSZ
        g = t // GT
        key = ((g * NBINS + b) * GT) + (t % GT)
        o = np.argsort(key, kind="stable")
        d, s, key = d[o], s[o], key[o]
        np.add.at(cnts[c], (t[o], b[o]), 1)
        cores.append((d, s, key))
    pb = cnts.max(axis=0)                      # [NT, NBINS] exact max

    # static slot layout in (group, bin, tile) order
    cell_off = np.zeros((NT, NBINS), np.int64)
    gcalls, gslot0, gchunks = [], [], []
    slot_tile = []                             # per slot: owning tile
    off = 0
    for (t0, t1) in groups:
        calls = []
        gslot0.append(off)
        for b in range(NBINS):
            coff = off
            for t in range(t0, t1):
                cell_off[t][b] = off
                n = int(pb[t][b])
                if n:
                    slot_tile.append(np.full(n, t, np.int32))
                off += n
            n_call = off - coff
            pad = (-n_call) % 128
            if pad:
                slot_tile.append(np.full(pad, -1, np.int32))
                off += pad
            calls.append((coff, n_call + pad))
        gcalls.append(calls)
        gchunks.append((off - gslot0[-1]) // 128)
    total_slots = off
    slot_tile = np.concatenate(slot_tile)

    # static chunk-matmul schedule
    mms = []
    s_col = 0
    for g, (t0, t1) in enumerate(groups):
        glo = gslot0[g]
        gm = []
        for t in range(t0, t1):
            for b in range(NBINS):
                lo, n = int(cell_off[t][b]), int(pb[t][b])
                if n == 0:
                    continue
                c0 = (lo - glo) // 128
                c1 = -(-(lo + n - glo) // 128)
                for q in range(c0, c1):
                    gm.append((t, q, s_col))
                    s_col += 1
        if not gm:
            gm.append((t0, 0, s_col))
            s_col += 1
        mms.append(gm)
    total_mm = s_col
    meta = dict(groups=groups, gcalls=gcalls, gslot0=gslot0,
                gchunks=gchunks, mms=mms,
                nmm_max=max(len(gm) for gm in mms),
                ch_max=max(gchunks), total_slots=total_slots,
                total_mm=total_mm)

    # per-core tables
    import ml_dtypes
    ngbt = ngroups * NBINS * GT
    mm_chunk = np.empty(total_mm, np.int64)
    mm_tile = np.empty(total_mm, np.int64)
    for g in range(ngroups):
        base = gslot0[g] // 128
        for (t_, q, sc) in mms[g]:
            mm_chunk[sc] = base + q
            mm_tile[sc] = t_
    T = slot_tile.reshape(-1, 128)
    percore = []
    for c in range(NCORES):
        d, s, key = cores[c]
        first = np.zeros(ngbt, np.int64)
        np.add.at(first, key, 1)
        cstart = np.concatenate([[0], np.cumsum(first)[:-1]])
        rank = np.arange(len(key)) - cstart[key]
        t = d // TW
        b = s // BINSZ
        pos = cell_off[t, b] + rank
        idx = np.zeros(total_slots, np.int16)
        gd = np.full(total_slots, -1, np.int64)     # absolute core-rel dst
        idx[pos] = (s - b * BINSZ).astype(np.int16)
        gd[pos] = d
        idx16 = np.tile(idx.reshape(-1, 16).T, (8, 1)).copy()

        # S stream: [total_mm, 128, TW] -> [128, total_mm * TW] fp8
        G = gd.reshape(-1, 128)
        rows_d = G[mm_chunk]                        # [nmm, 128]
        rows_t = T[mm_chunk]
        rel = rows_d - mm_tile[:, None] * TW
        valid = (rows_t == mm_tile[:, None]) & (rows_d >= 0)
        rel = np.where(valid, rel, -1)
        S = (rel[:, :, None] ==
             np.arange(TW, dtype=np.int64)[None, None, :])
        sfp8 = np.ascontiguousarray(
            S.transpose(1, 0, 2).reshape(128, total_mm * TW)).astype(
                ml_dtypes.float8_e4m3)
        percore.append(dict(idx16=idx16, sfp8=sfp8))
    return meta, percore


def _build(nc: bass.Bass, mu: dict, mi: dict):
    vtab = nc.dram_tensor("vtab", [N_NODES, D], BF16, kind="ExternalInput")
    utab = nc.dram_tensor("utab", [N_NODES, D], BF16, kind="ExternalInput")
    ufeaT = nc.dram_tensor("ufeaT", [128, AGG_ROWS], BF16, kind="ExternalInput")
    vfeaT = nc.dram_tensor("vfeaT", [128, AGG_ROWS], BF16, kind="ExternalInput")
    gidx_u = nc.dram_tensor("gidx_u", [128, mu["total_slots"] // 16], I16,
                            kind="ExternalInput")
    gidx_i = nc.dram_tensor("gidx_i", [128, mi["total_slots"] // 16], I16,
                            kind="ExternalInput")
    s_u = nc.dram_tensor("s_u", [128, mu["total_mm"] * TW], FP8,
                         kind="ExternalInput")
    s_i = nc.dram_tensor("s_i", [128, mi["total_mm"] * TW], FP8,
                         kind="ExternalInput")
    wn = {}
    for w in ("W1", "W2", "Wu_t", "Wu_b", "Wi_t", "Wi_b"):
        wn[w] = nc.dram_tensor(w, [128, 128], BF16, kind="ExternalInput")
    for b in ("b1", "b2", "bu", "bi"):
        wn[b] = nc.dram_tensor(b, [128, 1], F32, kind="ExternalInput")

    unT = nc.dram_tensor("unT", [128, AGG_ROWS], BF16, kind="ExternalOutput")
    uhT = nc.dram_tensor("uhT", [128, AGG_ROWS], BF16, kind="ExternalOutput")
    inT = nc.dram_tensor("inT", [128, AGG_ROWS], BF16, kind="ExternalOutput")
    ihT = nc.dram_tensor("ihT", [128, AGG_ROWS], BF16, kind="ExternalOutput")

    ch_max = max(mu["ch_max"], mi["ch_max"])
    nmm_max = max(mu["nmm_max"], mi["nmm_max"])

    with TileContext(nc) as tc:
        with (
            tc.tile_pool(name="wts", bufs=1) as wtsp,
            tc.tile_pool(name="idx", bufs=1) as idxp,
            tc.tile_pool(name="msg", bufs=3) as msgp,
            tc.tile_pool(name="stp", bufs=3) as stpp,
            tc.tile_pool(name="ftp", bufs=8) as ftpp,
            tc.tile_pool(name="cmp", bufs=4) as cmpp,
            tc.tile_pool(name="ntp", bufs=4) as ntpp,
            tc.tile_pool(name="htp", bufs=3) as htpp,
            tc.tile_pool(name="agu", bufs=2, space="PSUM") as aggpU,
            tc.tile_pool(name="agi", bufs=2, space="PSUM") as aggpI,
            tc.tile_pool(name="pnp", bufs=2, space="PSUM") as pnpp,
            tc.tile_pool(name="php", bufs=2, space="PSUM") as phpp,
        ):
            w = {}
            for name in ("W1", "W2", "Wu_t", "Wu_b", "Wi_t", "Wi_b"):
                w[name] = wtsp.tile([128, 128], BF16, tag=name, name=f"w_{name}")
                nc.sync.dma_start(w[name][:], wn[name][:])
            for name in ("b1", "b2", "bu", "bi"):
                w[name] = wtsp.tile([128, 1], F32, tag=name, name=f"w_{name}")
                nc.sync.dma_start(w[name][:], wn[name][:])
            t_alpha = wtsp.tile([128, 1], F32, tag="alpha")
            nc.vector.memset(t_alpha[:], ALPHA)

            t_gi = {}
            for key, gi, m in (("u", gidx_u, mu), ("i", gidx_i, mi)):
                t_gi[key] = idxp.tile([128, m["total_slots"] // 16], I16,
                                      tag=f"gi{key}", name=f"t_gi_{key}")
                nc.sync.dma_start(t_gi[key][:], gi[:])

            class Dir:
                def __init__(self, key, table, m, feaT, W1n, b1n, Wtn, Wbn,
                             btn, nT_out, hT_out, aggp):
                    self.key, self.table, self.m = key, table, m
                    self.feaT = feaT
                    self.W1n, self.b1n = W1n, b1n
                    self.Wtn, self.Wbn, self.btn = Wtn, Wbn, btn
                    self.nT_out, self.hT_out = nT_out, hT_out
                    self.aggp = aggp
                    self.pend = []      # [t, psA, ft] -> +nT after stageA
                    self.scol0 = 0

            du = Dir("u", vtab, mu, ufeaT, "W1", "b1", "Wu_t", "Wu_b", "bu",
                     unT, uhT, aggpU)
            di = Dir("i", utab, mi, vfeaT, "W2", "b2", "Wi_t", "Wi_b", "bi",
                     inT, ihT, aggpI)
            s_dram = {"u": s_u, "i": s_i}

            def stageA(dd, ent):
                t, psA, ft = ent
                j0 = t * TW
                aggT = cmpp.tile([128, TW], BF16, tag="aggT")
                nc.vector.tensor_copy(aggT[:], psA[:])
                pn = pnpp.tile([128, TW], F32, tag="pn")
                nc.tensor.matmul(pn[:], w[dd.W1n][:], aggT[:],
                                 start=True, stop=True)
                nT = ntpp.tile([128, TW], BF16, tag="nT")
                nc.scalar.activation(
                    nT[:], pn[:], mybir.ActivationFunctionType.Prelu,
                    bias=w[dd.b1n][:], scale=1.0, alpha=t_alpha[:])
                nc.sync.dma_start(dd.nT_out[:, j0:j0 + TW], nT[:])
                ent.append(nT)

            def stageB(dd, ent):
                t, psA, ft, nT = ent
                j0 = t * TW
                ph = phpp.tile([128, TW], F32, tag="ph")
                nc.tensor.matmul(ph[:], w[dd.Wtn][:], ft[:],
                                 start=True, stop=False)
                nc.tensor.matmul(ph[:], w[dd.Wbn][:], nT[:],
                                 start=False, stop=True)
                hT = htpp.tile([128, TW], BF16, tag="hT")
                nc.scalar.activation(
                    hT[:], ph[:], mybir.ActivationFunctionType.Relu,
                    bias=w[dd.btn][:], scale=1.0)
                nc.sync.dma_start(dd.hT_out[:, j0:j0 + TW], hT[:])

            def group(dd, g):
                m, gi = dd.m, t_gi[dd.key]
                t0, t1 = m["groups"][g]
                gm = m["mms"][g]
                nmm = len(gm)
                glo = m["gslot0"][g]
                mt = msgp.tile([128, ch_max, 128], BF16, tag="mt")
                stt = stpp.tile([128, nmm_max, TW], FP8, tag="stt")
                nc.sync.dma_start(
                    stt[:, :nmm, :],
                    s_dram[dd.key][:, dd.scol0 * TW:(dd.scol0 + nmm) * TW])
                for b in range(NBINS):
                    ioff, nidx = m["gcalls"][g][b]
                    if nidx == 0:
                        continue
                    bhi = min(N_NODES, (b + 1) * BINSZ)
                    nc.gpsimd.dma_gather(
                        out_ap=mt[:, (ioff - glo) // 128:
                                  (ioff - glo + nidx) // 128, :],
                        in_ap=dd.table[b * BINSZ:bhi, :],
                        idxs_ap=gi[:, ioff // 16:(ioff + nidx) // 16],
                        num_idxs=nidx,
                        num_idxs_reg=nidx,
                        elem_size=D,
                        single_packet=False,
                        queue_num=b,
                    )
                for t in range(t0, t1):
                    psA = dd.aggp.tile([128, TW], F32, tag="psA")
                    runs = [x for x in gm if x[0] == t]
                    for j, (_, q, sc) in enumerate(runs):
                        nc.tensor.matmul(
                            psA[:], mt[:, q, :],
                            stt[:, sc - dd.scol0, :],
                            start=(j == 0), stop=(j == len(runs) - 1))
                    ft = ftpp.tile([128, TW], BF16, tag="ft")
                    nc.sync.dma_start(ft[:], dd.feaT[:, t * TW:(t + 1) * TW])
                    dd.pend.append([t, psA, ft])
                    if len(dd.pend) >= 2:
                        stageA(dd, dd.pend[-2])
                    if len(dd.pend) >= 3:
                        stageB(dd, dd.pend.pop(0))
                dd.scol0 += nmm

            def drain(dd):
                if dd.pend:
                    stageA(dd, dd.pend[-1])
                for ent in dd.pend:
                    stageB(dd, ent)
                dd.pend = []

            ngu, ngi = len(mu["groups"]), len(mi["groups"])
            with nc.named_scope("main"):
                for g in range(max(ngu, ngi)):
                    if g < ngu:
                        group(du, g)
                    if g < ngi:
                        group(di, g)
                drain(du)
                drain(di)
    return nc


def _run(ufea, vfea, uv_rows, uv_cols):
    mu, pcu = _prep_direction(uv_rows, uv_cols)
    mi, pci = _prep_direction(uv_cols, uv_rows)

    nc = bacc.Bacc("TRN2", target_bir_lowering=False, debug=False,
                   dynamic_dma_scratch_size=2**16, num_swdge_queues=4)
    _build(nc, mu, mi)
    nc.compile()

    import ml_dtypes
    bf = ml_dtypes.bfloat16
    common = {
        "vtab": vfea.astype(bf),
        "utab": ufea.astype(bf),
    }
    in_maps = []
    for c in range(NCORES):
        m = dict(common)
        fu = np.zeros((128, AGG_ROWS), np.float32)
        fv = np.zeros((128, AGG_ROWS), np.float32)
        fu[:, :SH] = ufea[c * SH:(c + 1) * SH].T
        fv[:, :SH] = vfea[c * SH:(c + 1) * SH].T
        m["ufeaT"] = fu.astype(bf)
        m["vfeaT"] = fv.astype(bf)
        m["gidx_u"] = pcu[c]["idx16"]
        m["s_u"] = pcu[c]["sfp8"]
        m["gidx_i"] = pci[c]["idx16"]
        m["s_i"] = pci[c]["sfp8"]
        in_maps.append(m)
    return nc, in_maps


def kernel(ufea, vfea, uv_rows, uv_cols, uv_vals,
           W1, b1, W2, b2, Wu, bu, Wi, bi) -> np.ndarray:
    import ml_dtypes
    bf = ml_dtypes.bfloat16
    ufea = np.ascontiguousarray(np.asarray(ufea, np.float32))
    vfea = np.ascontiguousarray(np.asarray(vfea, np.float32))
    uv_rows = np.asarray(uv_rows, np.int32)
    uv_cols = np.asarray(uv_cols, np.int32)

    nc, in_maps = _run(ufea, vfea, uv_rows, uv_cols)
    Wu = np.asarray(Wu, np.float32)
    Wi = np.asarray(Wi, np.float32)
    for m in in_maps:
        m.update({
            "W1": np.asarray(W1, np.float32).astype(bf),
            "W2": np.asarray(W2, np.float32).astype(bf),
            "Wu_t": np.ascontiguousarray(Wu[:128]).astype(bf),
            "Wu_b": np.ascontiguousarray(Wu[128:]).astype(bf),
            "Wi_t": np.ascontiguousarray(Wi[:128]).astype(bf),
            "Wi_b": np.ascontiguousarray(Wi[128:]).astype(bf),
            "b1": np.asarray(b1, np.float32).reshape(128, 1),
            "b2": np.asarray(b2, np.float32).reshape(128, 1),
            "bu": np.asarray(bu, np.float32).reshape(128, 1),
            "bi": np.asarray(bi, np.float32).reshape(128, 1),
        })

    res = bass_utils.run_bass_kernel_spmd(nc, in_maps, list(range(NCORES)),
                                          trace=False)

    out = np.empty((6, N_NODES, D), np.float32)
    for c in range(NCORES):
        r = res.results[c]
        sl = slice(c * SH, (c + 1) * SH)
        out[0][sl] = r["uhT"][:, :SH].T.astype(np.float32)
        out[1][sl] = r["unT"][:, :SH].T.astype(np.float32)
        out[3][sl] = r["ihT"][:, :SH].T.astype(np.float32)
        out[4][sl] = r["inT"][:, :SH].T.astype(np.float32)
    out[2] = ufea
    out[5] = vfea
    return out
